# revision 3
# baseline (speedup 1.0000x reference)
# Pairwise Euclidean distance kernel for Trainium2 (Bass/Tile).
#
# Input : coordinates_batch [8, 2048, 3] f32
# Output: [8, 2048, 2048] f32, out[b,i,j] = ||c[b,i] - c[b,j]||
#
# Sharding: data-parallel over batch -- one batch element per NeuronCore (8 cores).
#
# Per-core algorithm: dist^2(i,j) = |ci|^2 + |cj|^2 - 2 ci.cj computed as a single
# K=5 augmented matmul on the tensor engine:
#   L = [x; y; z; |c|^2; 1]       (lhsT, [5, N], prepared host-side, 40 KB)
#   R = [-2x; -2y; -2z; 1; |c|^2] (rhs,  [5, N], prepared host-side, 40 KB)
#   (L[:, I].T @ R[:, J])[i, j] = dist^2(i, j)
# then clamp to >= 0 on the vector engine (fp32 cancellation can go slightly
# negative near the diagonal), sqrt on the scalar engine, exact-zero the
# diagonal via affine_select on gpsimd, and DMA each [128, 2048] row block out
# (1 MiB contiguous per transfer).

import numpy as np

B, N, D = 8, 2048, 3
K = 5            # augmented contraction dim
P = 128          # output row tile (partition dim)
FT = 512         # psum free-dim tile (one PSUM bank of f32)
NI = N // P      # 16 row tiles
NJ = N // FT     # 4 column tiles per row

_cached_nc = None


def _build_nc():
    global _cached_nc
    if _cached_nc is not None:
        return _cached_nc

    import concourse.bacc as bacc
    import concourse.mybir as mybir
    import concourse.tile as tile

    nc = bacc.Bacc("TRN2", target_bir_lowering=False, debug=False)
    Ld = nc.dram_tensor("L", [K, N], mybir.dt.float32, kind="ExternalInput")
    Rd = nc.dram_tensor("R", [K, N], mybir.dt.float32, kind="ExternalInput")
    out = nc.dram_tensor("out", [N, N], mybir.dt.float32, kind="ExternalOutput")

    f32 = mybir.dt.float32

    with tile.TileContext(nc) as tc:
        with (
            tc.tile_pool(name="singles", bufs=1) as singles,
            tc.tile_pool(name="rows", bufs=4) as rows,
            tc.tile_pool(name="psum", bufs=8, space="PSUM") as psum,
        ):
            L = singles.tile([K, N], f32)
            R = singles.tile([K, N], f32)
            nc.sync.dma_start(out=L, in_=Ld.ap())
            nc.sync.dma_start(out=R, in_=Rd.ap())

            for it in range(NI):
                row = rows.tile([P, N], f32)
                for jt in range(NJ):
                    js = slice(jt * FT, (jt + 1) * FT)
                    ps = psum.tile([P, FT], f32)
                    nc.tensor.matmul(
                        ps,
                        lhsT=L[:, it * P : (it + 1) * P],
                        rhs=R[:, js],
                        start=True,
                        stop=True,
                    )
                    nc.vector.tensor_scalar_max(row[:, js], ps, 0.0)
                    nc.scalar.sqrt(row[:, js], row[:, js])
                # exact-zero the diagonal of the diagonal block: keep where
                # (free_idx - partition_idx) != 0
                dslice = slice(it * P, (it + 1) * P)
                nc.gpsimd.affine_select(
                    out=row[:, dslice],
                    in_=row[:, dslice],
                    pattern=[[1, P]],
                    compare_op=mybir.AluOpType.not_equal,
                    fill=0.0,
                    base=0,
                    channel_multiplier=-1,
                )
                nc.sync.dma_start(out=out[it * P : (it + 1) * P, :], in_=row)

    nc.compile()
    _cached_nc = nc
    return nc


def _augment(x: np.ndarray):
    """x: [B, N, 3] f32 -> (L [B, 5, N], R [B, 5, N]) f32."""
    xt = np.transpose(x, (0, 2, 1)).astype(np.float32)          # [B, 3, N]
    n2 = np.sum(x.astype(np.float64) ** 2, axis=2, dtype=np.float64)  # [B, N]
    n2 = n2.astype(np.float32)
    ones = np.ones((x.shape[0], 1, x.shape[1]), np.float32)
    L = np.concatenate([xt, n2[:, None, :], ones], axis=1)      # [B, 5, N]
    R = np.concatenate([-2.0 * xt, ones, n2[:, None, :]], axis=1)
    return np.ascontiguousarray(L), np.ascontiguousarray(R)


def run(coordinates_batch: np.ndarray, trace: bool = False):
    """Run on 8 NeuronCores; returns (output [8,2048,2048] f32, BassKernelResults)."""
    from concourse.bass_utils import run_bass_kernel_spmd

    nc = _build_nc()
    x = np.ascontiguousarray(np.asarray(coordinates_batch), dtype=np.float32)
    assert x.shape == (B, N, D), x.shape
    L, R = _augment(x)
    in_maps = [{"L": L[b], "R": R[b]} for b in range(B)]
    res = run_bass_kernel_spmd(nc, in_maps, core_ids=list(range(B)), trace=trace)
    out = np.stack([r["out"] for r in res.results], axis=0)
    return out, res


def kernel(coordinates_batch: np.ndarray) -> np.ndarray:
    out, _ = run(coordinates_batch, trace=False)
    return out


# revision 4
# speedup vs baseline: 2.0982x; 2.0982x over previous
# Pairwise Euclidean distance kernel for Trainium2 (Bass/Tile).
#
# Input : coordinates_batch [8, 2048, 3] f32
# Output: [8, 2048, 2048] f32, out[b,i,j] = ||c[b,i] - c[b,j]||
#
# Sharding: data-parallel over batch -- one batch element per NeuronCore (8 cores).
#
# Per-core algorithm: dist^2(i,j) = |ci|^2 + |cj|^2 - 2 ci.cj.
# The cross term and |cj|^2 ride a single K=21 bf16 matmul per output tile:
# each fp32 row of the augmented matrices is 3-way bf16-split (h + m + l) and
# the product keeps the 6 significant digit-pair groups
#   (h,h),(m,h),(h,m),(l,h),(h,l),(m,m)
# stacked along K (3 coord rows per group = 18 rows) plus 3 rows (1 x n2_{h,m,l})
# for |cj|^2 -- residual ~2^-27, i.e. fp32-grade accuracy at bf16 streaming
# speed (1 cycle/column on the PE instead of 8 for fp32 x fp32).
# |ci|^2 is added EXACTLY in fp32 by the vector engine via the per-partition
# scalar operand of the clamp op: row = max(psum + n2_i, 0).
# Then sqrt on the scalar engine (in place), exact-zero of the diagonal via
# affine_select on gpsimd, and one contiguous 1 MiB DMA per 128-row block.

import numpy as np

B, N, D = 8, 2048, 3
K = 21           # stacked bf16-split contraction dim
P = 128          # output row tile (partition dim)
FT = 512         # matmul free-dim chunk (one PSUM bank of f32)
NI = N // P      # 16 row tiles
NJ = N // FT     # 4 matmul chunks per row tile

_cached_nc = None


def _build_nc():
    global _cached_nc
    if _cached_nc is not None:
        return _cached_nc

    import concourse.bacc as bacc
    import concourse.mybir as mybir
    import concourse.tile as tile

    nc = bacc.Bacc("TRN2", target_bir_lowering=False, debug=False)
    bf16 = mybir.dt.bfloat16
    f32 = mybir.dt.float32

    Ld = nc.dram_tensor("lhs", [K, N], bf16, kind="ExternalInput")
    Rd = nc.dram_tensor("rhs", [K, N], bf16, kind="ExternalInput")
    Nd = nc.dram_tensor("n2t", [P, NI], f32, kind="ExternalInput")
    out = nc.dram_tensor("out", [N, N], f32, kind="ExternalOutput")

    with tile.TileContext(nc) as tc:
        with (
            tc.tile_pool(name="singles", bufs=1) as singles,
            tc.tile_pool(name="rows", bufs=4) as rows,
            tc.tile_pool(name="psum", bufs=2, space="PSUM") as psum,
        ):
            L = singles.tile([K, N], bf16)
            R = singles.tile([K, N], bf16)
            n2t = singles.tile([P, NI], f32)
            nc.sync.dma_start(out=L, in_=Ld.ap())
            nc.sync.dma_start(out=R, in_=Rd.ap())
            nc.sync.dma_start(out=n2t, in_=Nd.ap())

            for it in range(NI):
                row = rows.tile([P, N], f32)
                ps = psum.tile([P, N], f32)  # 4 PSUM banks
                for jt in range(NJ):
                    js = slice(jt * FT, (jt + 1) * FT)
                    nc.tensor.matmul(
                        ps[:, js],
                        lhsT=L[:, it * P : (it + 1) * P],
                        rhs=R[:, js],
                        start=True,
                        stop=True,
                    )
                # row = max(psum + n2_i, 0)   (n2_i exact in fp32)
                nc.vector.tensor_scalar(
                    out=row,
                    in0=ps,
                    scalar1=n2t[:, it : it + 1],
                    scalar2=0.0,
                    op0=mybir.AluOpType.add,
                    op1=mybir.AluOpType.max,
                )
                nc.scalar.sqrt(row, row)
                # exact-zero the diagonal of the diagonal block: keep where
                # (free_idx - partition_idx) != 0
                dslice = slice(it * P, (it + 1) * P)
                nc.gpsimd.affine_select(
                    out=row[:, dslice],
                    in_=row[:, dslice],
                    pattern=[[1, P]],
                    compare_op=mybir.AluOpType.not_equal,
                    fill=0.0,
                    base=0,
                    channel_multiplier=-1,
                )
                nc.sync.dma_start(out=out[it * P : (it + 1) * P, :], in_=row)

    nc.compile()
    _cached_nc = nc
    return nc


def _augment(x: np.ndarray):
    """x: [B, N, 3] f32 -> (lhsT [B,21,N] bf16, rhs [B,21,N] bf16, n2t [B,128,16] f32)."""
    import ml_dtypes

    bf = ml_dtypes.bfloat16

    def split3(a):
        h = a.astype(bf).astype(np.float32)
        r = a - h
        m = r.astype(bf).astype(np.float32)
        l = (r - m).astype(bf).astype(np.float32)
        return h, m, l

    nb = x.shape[0]
    xt = np.transpose(x, (0, 2, 1)).astype(np.float32)           # [B,3,N]
    n2 = np.sum(x.astype(np.float64) ** 2, axis=2).astype(np.float32)  # [B,N]
    m2 = (-2.0 * x.astype(np.float64)).astype(np.float32).transpose(0, 2, 1)  # [B,3,N]

    ch, cm, cl = split3(xt)
    mh, mm, ml = split3(m2)
    nh, nm, nl = split3(n2[:, None, :])
    one = np.ones((nb, 1, x.shape[1]), np.float32)

    lhsT = np.concatenate([ch, cm, ch, cl, ch, cm, one, one, one], 1)  # [B,21,N]
    rhs = np.concatenate([mh, mh, mm, mh, ml, mm, nh, nm, nl], 1)      # [B,21,N]
    n2t = np.transpose(n2.reshape(nb, NI, P), (0, 2, 1))               # [B,128,16]
    return (
        np.ascontiguousarray(lhsT.astype(bf)),
        np.ascontiguousarray(rhs.astype(bf)),
        np.ascontiguousarray(n2t.astype(np.float32)),
    )


def run(coordinates_batch: np.ndarray, trace: bool = False):
    """Run on 8 NeuronCores; returns (output [8,2048,2048] f32, BassKernelResults)."""
    from concourse.bass_utils import run_bass_kernel_spmd

    nc = _build_nc()
    x = np.ascontiguousarray(np.asarray(coordinates_batch), dtype=np.float32)
    assert x.shape == (B, N, D), x.shape
    lhsT, rhs, n2t = _augment(x)
    in_maps = [{"lhs": lhsT[b], "rhs": rhs[b], "n2t": n2t[b]} for b in range(B)]
    res = run_bass_kernel_spmd(nc, in_maps, core_ids=list(range(B)), trace=trace)
    out = np.stack([r["out"] for r in res.results], axis=0)
    return out, res


def kernel(coordinates_batch: np.ndarray) -> np.ndarray:
    out, _ = run(coordinates_batch, trace=False)
    return out
